# revision 29
# baseline (speedup 1.0000x reference)
"""Multi-head attention (b=2, t=2048, d=1024, h=16, hd=64) on 8 trn2 NeuronCores.

Sharding: core c = 4*b + g handles batch b and head-group g (4 heads,
feature columns [g*256, (g+1)*256)). QKV weights column-sharded, Wo
row-sharded (Megatron); each core returns ONE fp16 partial [2048, 1024]
(both 128-row Wo chunks accumulated on-device in PSUM); the host sums
the 4 group partials per batch and adds bo.

Datapath: fp16 operands everywhere on the PE (x, Wq/Wk/Wv, Q^T, K^T, V,
probs, ctx, Wo) with fp32 PSUM accumulation. Softmax skips
max-subtraction: scores are q.k/8 with q,k ~ N(0,1), far inside exp's
range.

Schedule: 8 attention phases of (fb, q-quarter): the two 64-row heads of
feature-block fb are processed CONCURRENTLY via PE row-tiles (partitions
0-63 / 64-127 -> tile_position (0,0)/(64,0)) into the two 512-wide
halves of one [128,1024] PSUM tile, so each (sb, phase) step is a single
[128,1024] exp ACTIVATE. ACT(exp) is the roofline (~147us); all other PE
work (Q/K projections, token-major V projection, fused fb0+fb1 output
projection) is interleaved as fillers to keep the PE HAM clock-gate warm
and hidden under ACT. Softmax denominators are broadcast by a K=1
matmul and inverted with the fast custom-DVE reciprocal.
"""

import numpy as np

import concourse.bass as bass
import concourse.mybir as mybir
import concourse.tile as tile
from concourse.bass_utils import run_bass_kernel_spmd

F32 = mybir.dt.float32
F32R = mybir.dt.float32r
F16 = mybir.dt.float16
EXP = mybir.ActivationFunctionType.Exp

T = 2048          # tokens per batch
D = 1024          # model dim
HG = 4            # heads per core
HD = 64           # head dim
GF = HG * HD      # 256 features per head-group
VW = HG * (HD + 1)  # 260: V columns + a ones column per head
NT = T // 128     # 16 token blocks

MAX_WAITS = 1


def _split_waits(nc):
    """walrus in this container allows only one sync-wait per instruction;
    hoist extras onto same-engine NoOps immediately before the offender."""
    for f in nc.m.functions:
        for blk in f.blocks:
            insts = list(blk.instructions)
            new, changed = [], False
            for ins in insts:
                si = ins.sync_info
                waits = list(si.on_wait) if si and si.on_wait else []
                if len(waits) > MAX_WAITS:
                    changed = True
                    extra, keep = waits[:-MAX_WAITS], waits[-MAX_WAITS:]
                    for i in range(0, len(extra), MAX_WAITS):
                        new.append(mybir.InstNoOp(
                            name=f"{ins.name}-wsplit{i}",
                            engine=ins.engine,
                            sync_info=mybir.SyncInfo(
                                on_wait=extra[i:i + MAX_WAITS], on_update=[]),
                        ))
                    ins.sync_info = mybir.SyncInfo(
                        on_wait=keep,
                        on_update=list(si.on_update) if si.on_update else [])
                new.append(ins)
            if changed:
                blk.instructions = new


def _build_program():
    nc = bass.Bass("TRN2", target_bir_lowering=False, debug=False, num_devices=8)

    xT = nc.dram_tensor("xT", [D, T], F16, kind="ExternalInput")
    Wq = nc.dram_tensor("Wq", [D, GF], F16, kind="ExternalInput")
    Wk = nc.dram_tensor("Wk", [D, GF], F16, kind="ExternalInput")
    Wv = nc.dram_tensor("Wv", [D, GF], F16, kind="ExternalInput")
    Wo = nc.dram_tensor("Wo", [GF, D], F16, kind="ExternalInput")
    bq = nc.dram_tensor("bq", [GF, 1], F32, kind="ExternalInput")
    bk = nc.dram_tensor("bk", [GF, 1], F32, kind="ExternalInput")
    bvr = nc.dram_tensor("bvr", [1, GF], F16, kind="ExternalInput")
    out = nc.dram_tensor("out", [T, D], F16, kind="ExternalOutput")

    with tile.TileContext(nc) as tc:
        with (
            nc.allow_low_precision(reason="fp16 rounding is intentional"),
            tc.tile_pool(name="w", bufs=1) as wp,       # persistent tiles
            tc.tile_pool(name="xt", bufs=8) as xp,      # xT tiles
            tc.tile_pool(name="pt", bufs=4) as ptp,     # probs tiles
            tc.tile_pool(name="sg", bufs=4) as sgp,     # stg / recip tiles
            tc.tile_pool(name="ob", bufs=3) as obp,     # out staging
            tc.tile_pool(name="ps", bufs=2, space="PSUM") as psA,   # scratch
            tc.tile_pool(name="pst", bufs=2, space="PSUM") as psT,  # S tiles
            tc.tile_pool(name="psc", bufs=2, space="PSUM") as psC,  # C accum
        ):
            # ---- tiny constants (no DMA dependency) ------------------------
            ones16 = wp.tile([1, 512], F16, tag="ones16")
            nc.gpsimd.memset(ones16[:], 1.0)
            # f32r ones row at partition 0 for the recip-broadcast matmul
            ones_f = wp.tile([1, 128], F32, tag="ones_f")
            nc.gpsimd.memset(ones_f[:], 1.0)
            onesr = wp.tile([1, 128], F32R, tag="onesr")
            nc.vector.tensor_copy(onesr[:], ones_f[:])
            wscr = wp.tile([1, 512], F16, tag="wscr")

            # warm the PE HAM clock + load the exp ACT table while input
            # DMAs land. K=1 matmuls do NOT drive the HAM activity monitor
            # past its un-throttle threshold — use full K=128 matmuls on a
            # memset tile.
            wbig = wp.tile([128, 512], F16, tag="wbig")
            nc.gpsimd.memset(wbig[:], 0.01)
            wps = psA.tile([128, 512], F32, tag="sp", name="warm")
            for i in range(36):
                nc.tensor.matmul(wps[:], wbig[:, 0:128], wbig[:],
                                 start=True, stop=True)
            nc.scalar.activation(wscr[:], wps[0:1, :], EXP, scale=0.0001)

            def keep_warm(n, pool=None):
                """dependency-free K=128 matmuls that keep the HAM clock at
                8/8 through stretches where real PE work is blocked on DMAs
                or evacs."""
                if pool is None:
                    w = psT.tile([128, 1024], F32, tag="st", name="warm2")
                    w = w[:, 0:512]
                else:
                    w = pool.tile([128, 512], F32, tag="sp", name="warm2")[:]
                for i in range(n):
                    nc.tensor.matmul(w, wbig[:, 0:128], wbig[:],
                                     start=True, stop=True)

            # ---- input DMAs: few big transfers, split across the two
            # HWDGE queues (sync + scalar) and gpsimd SWDGE ----------------
            def chunked(dst, src, c, f, eng, half=None):
                """load src [(c p), f] into dst [128, c*f] chunk-major."""
                o = dst[:].rearrange("p (c f) -> p c f", c=c, f=f)
                i = src[:, :].rearrange("(c p) f -> p c f", c=c, p=128)
                if half is not None:
                    h0 = half * (f // 2)
                    o = o[:, :, h0:h0 + f // 2]
                    i = i[:, :, h0:h0 + f // 2]
                eng.dma_start(o, i)

            bq_t, bk_t = [], []
            for fb in range(2):
                b = wp.tile([128, 1], F32, tag=f"bq{fb}", name=f"bq{fb}")
                nc.sync.dma_start(b[:], bq[fb * 128:(fb + 1) * 128, :])
                bq_t.append(b)
                b = wp.tile([128, 1], F32, tag=f"bk{fb}", name=f"bk{fb}")
                nc.scalar.dma_start(b[:], bk[fb * 128:(fb + 1) * 128, :])
                bk_t.append(b)

            Wkb = wp.tile([128, 8 * GF], F16, tag="wkb")
            chunked(Wkb, Wk, 8, GF, nc.scalar)
            # x arrives in token quarters, split across the two HWDGE
            # queues so projection units start as their quarter lands; Q/V
            # weights ride the gpsimd SWDGE queue.
            xb = wp.tile([128, 8 * T], F16, tag="xb")
            xbv = xb[:].rearrange("p (c f) -> p c f", c=8, f=T)
            xiv = xT[:, :].rearrange("(c p) f -> p c f", c=8, p=128)
            for qtr, eng in ((0, nc.sync), (1, nc.sync),
                             (2, nc.scalar), (3, nc.scalar)):
                qs = slice(qtr * 512, (qtr + 1) * 512)
                eng.dma_start(xbv[:, :, qs], xiv[:, :, qs])
            bv_t = wp.tile([1, GF], F16, tag="bvr")
            nc.gpsimd.dma_start(bv_t[:], bvr[:, :])
            Wvb = wp.tile([128, 8 * GF], F16, tag="wvb")
            chunked(Wvb, Wv, 8, GF, nc.gpsimd)
            Wqb = wp.tile([128, 8 * GF], F16, tag="wqb")
            chunked(Wqb, Wq, 8, GF, nc.gpsimd)
            Wob = wp.tile([128, 2 * D], F16, tag="wob")
            chunked(Wob, Wo, 2, D, nc.gpsimd)

            Wq_t = [Wqb[:, dc * GF:(dc + 1) * GF] for dc in range(8)]
            Wk_t = [Wkb[:, dc * GF:(dc + 1) * GF] for dc in range(8)]
            Wv_t = [Wvb[:, dc * GF:(dc + 1) * GF] for dc in range(8)]
            Wo_t = [Wob[:, fb * D:(fb + 1) * D] for fb in range(2)]
            xT_t = [xb[:, dc * T:(dc + 1) * T] for dc in range(8)]

            # ---- persistent SBUF tensors -----------------------------------
            QT = [wp.tile([128, T], F16, tag=f"qt{fb}", name=f"qt{fb}")
                  for fb in range(2)]
            KT = [wp.tile([128, T], F16, tag=f"kt{fb}", name=f"kt{fb}")
                  for fb in range(2)]
            # V in token-major blocks [128 tokens, 4*(64+1)]; col h*65+64
            # is a ones column so the ct psum row 64 is the softmax denom.
            V_t = [wp.tile([128, VW], F16, tag=f"v{tb}", name=f"v{tb}")
                   for tb in range(NT)]
            for tb in range(NT):
                vv = V_t[tb][:].rearrange("p (h w) -> p h w", h=HG, w=HD + 1)
                nc.gpsimd.memset(vv[:, :, HD:HD + 1], 1.0)
            # normalized ctx, feature-major per fb: [128 feat, 2048 q] fp16
            CTn = [wp.tile([128, T], F16, tag=f"ctn{fb}", name=f"ctn{fb}")
                   for fb in range(2)]

            # ---- helpers ---------------------------------------------------
            def qk_proj(w_t, b_t, dst, fb, tck, half=None, state={}):
                """project feature block fb of Q or K for token chunk tck.
                half=0/1 runs only dc 0-3 / 4-7 so a unit can be split
                across two filler slots (keeps each slot under the ACT
                budget)."""
                if half in (None, 0):
                    state[id(dst), fb, tck] = p = psA.tile(
                        [128, 512], F32, tag="sp", name="sp")
                else:
                    p = state.pop((id(dst), fb, tck))
                dcs = {None: range(8), 0: range(4), 1: range(4, 8)}[half]
                for dc in dcs:
                    nc.tensor.matmul(
                        p[:],
                        w_t[dc][:, fb * 128:(fb + 1) * 128],
                        xT_t[dc][:, tck * 512:(tck + 1) * 512],
                        start=(dc == 0), stop=(dc == 7))
                if half in (None, 1):
                    nc.vector.tensor_scalar_add(
                        dst[fb][:, tck * 512:(tck + 1) * 512], p[:], b_t[fb])

            def v_unit(tb):
                """token-major V projection for token block tb (+bias)."""
                p = psA.tile([128, 512], F32, tag="sp", name="sp")
                nc.tensor.matmul(p[:, 0:GF], ones16[0:1, 0:128], bv_t[0:1, :],
                                 start=True, stop=False)
                for dc in range(8):
                    nc.tensor.matmul(
                        p[:, 0:GF],
                        xT_t[dc][:, tb * 128:(tb + 1) * 128],
                        Wv_t[dc],
                        start=False, stop=(dc == 7))
                vv = V_t[tb][:].rearrange("p (h w) -> p h w", h=HG, w=HD + 1)
                pv = p[:, 0:GF].rearrange("p (h w) -> p h w", h=HG, w=HD)
                nc.vector.tensor_copy(vv[:, :, 0:HD], pv[:])

            def out_unit(tb, eng=None, nck_only=None, state={}):
                """fused fb0+fb1 output projection for token block tb.
                nck_only=0/1 runs a single 512-wide half per call."""
                if nck_only in (None, 0):
                    state[tb] = o = obp.tile([128, D], F16, tag="o", name="o")
                else:
                    o = state.pop(tb)
                for nck in ((0, 1) if nck_only is None else (nck_only,)):
                    p = psA.tile([128, 512], F32, tag="sp", name="sp")
                    for fb in range(2):
                        nc.tensor.matmul(
                            p[:],
                            CTn[fb][:, tb * 128:(tb + 1) * 128],
                            Wo_t[fb][:, nck * 512:(nck + 1) * 512],
                            start=(fb == 0), stop=(fb == 1))
                    nc.vector.tensor_copy(o[:, nck * 512:(nck + 1) * 512], p[:])
                if nck_only in (None, 1):
                    (eng or nc.sync).dma_start(
                        out[tb * 128:(tb + 1) * 128, :], o[:])

            def evac(ct, fb, hh, qq):
                """normalize ct [65, 512] into CTn[fb] rows hh*64..+64.

                The denominator row is DMA-transposed to [128, 4] so the
                slow DVE reciprocal runs 4 elems/lane instead of 512; the
                reciprocals are transposed back and broadcast to 64
                partitions by a K=1 matmul.
                """
                stg = sgp.tile([65, 512], F32R, tag="stg", name="stg")
                nc.vector.tensor_copy(stg[:], ct[:])
                dT = sgp.tile([128, 4], F32, tag="dT", name="dT")
                nc.sync.dma_start(
                    dT[:],
                    stg[64:65, :].bitcast(F32).rearrange(
                        "p (a b) -> p a b", a=128, b=4))
                rT = sgp.tile([128, 4], F32, tag="rT", name="rT")
                nc.vector.reciprocal(rT[:], dT[:])
                rrow = sgp.tile([1, 512], F32R, tag="rrow", name="rrow")
                nc.sync.dma_start(
                    rrow[:].bitcast(F32).rearrange(
                        "p (a b) -> p a b", a=128, b=4),
                    rT[:])
                rp = psA.tile([128, 512], F32, tag="sp", name="sp")
                nc.tensor.matmul(rp[:], onesr[0:1, :], rrow[0:1, :],
                                 start=True, stop=True)
                nc.vector.tensor_mul(
                    CTn[fb][hh * 64:(hh + 1) * 64,
                            qq * 512:(qq + 1) * 512],
                    stg[0:64, :], rp[0:64, :])

            # ---- pre-phase: only what attention phase (0,0) needs -- all
            # of K(fb0), Q(fb0) quarter 0, and the first V blocks -- ordered
            # to match the token-quarter DMA arrival. Dependency-free warm
            # matmuls keep the PE clock at 8/8 across the DMA-wait bubbles.
            qk_proj(Wk_t, bk_t, KT, 0, 0)
            keep_warm(3)
            v_unit(0)
            qk_proj(Wk_t, bk_t, KT, 0, 1)
            keep_warm(3)
            v_unit(1)
            v_unit(2)
            qk_proj(Wk_t, bk_t, KT, 0, 2)
            keep_warm(3)
            qk_proj(Wk_t, bk_t, KT, 0, 3)
            qk_proj(Wq_t, bq_t, QT, 0, 0)

            # ---- attention: 2 fb x 4 q-quarters, 2 heads row-tiled --------
            # extra fillers keyed by (fb, qq): {slot: callable}. Remaining
            # projection work and the output projection are split into
            # half-units so no slot exceeds the ~1.15us ACT budget (one
            # oversized filler slot stalls the 1-deep scores lookahead and
            # puts a bubble in the exp chain). The previous phase's two
            # evacs are always placed lazily at slots 1 and 3 so their DMA
            # round-trip never stalls the PE FIFO at a phase boundary.
            def qk2(w_t, b_t, dst, fb, tck, slots):
                return {s: (lambda h=h: qk_proj(w_t, b_t, dst, fb, tck, h))
                        for h, s in enumerate(slots)}

            def out2(tb, slots):
                return {s: (lambda n=n, tb=tb: out_unit(tb, nck_only=n))
                        for n, s in enumerate(slots)}

            def merge(*ds):
                r = {}
                for dd in ds:
                    for k, v in dd.items():
                        assert k not in r, f"slot clash {k}"
                        r[k] = v
                return r

            extra = {
                (0, 0): merge({j: (lambda j=j: v_unit(j + 3))
                               for j in range(12)},
                              {12: lambda: qk_proj(Wq_t, bq_t, QT, 0, 1),
                               13: lambda: v_unit(15)}),
                (0, 1): merge(qk2(Wq_t, bq_t, QT, 0, 2, (2, 4)),
                              qk2(Wk_t, bk_t, KT, 1, 0, (6, 8)),
                              qk2(Wk_t, bk_t, KT, 1, 1, (10, 12))),
                (0, 2): merge(qk2(Wq_t, bq_t, QT, 0, 3, (2, 4)),
                              qk2(Wq_t, bq_t, QT, 1, 0, (6, 8)),
                              qk2(Wk_t, bk_t, KT, 1, 2, (10, 12)),
                              qk2(Wk_t, bk_t, KT, 1, 3, (13, 15))),
                (0, 3): qk2(Wq_t, bq_t, QT, 1, 1, (2, 4)),
                (1, 0): merge(qk2(Wq_t, bq_t, QT, 1, 2, (2, 4)),
                              qk2(Wq_t, bq_t, QT, 1, 3, (6, 8))),
                (1, 1): merge(out2(0, (5, 6)), out2(1, (7, 9)),
                              out2(2, (10, 12)), out2(3, (13, 14))),
                (1, 2): merge(out2(4, (5, 6)), out2(5, (7, 9)),
                              out2(6, (10, 12)), out2(7, (13, 14))),
                (1, 3): merge(out2(8, (5, 6)), out2(9, (7, 9)),
                              out2(10, (10, 12)), out2(11, (13, 14))),
            }

            prev = None
            for fb in range(2):
                for qq in range(4):
                    qw = slice(qq * 512, (qq + 1) * 512)
                    cts = [psC.tile([65, 512], F32, tag="ct", name=f"ct{hh}")
                           for hh in range(2)]
                    pts = {}
                    fl = dict(extra.get((fb, qq), {}))
                    if prev is not None:
                        pc, pf, pq = prev
                        fl[1] = (lambda a=fl.get(1), pc=pc, pf=pf, pq=pq:
                                 (evac(pc[0], pf, 0, pq), a and a()))
                        fl[3] = (lambda a=fl.get(3), pc=pc, pf=pf, pq=pq:
                                 (evac(pc[1], pf, 1, pq), a and a()))

                    def c_mms(j, cts=cts, fb=fb, pts=pts):
                        for hh in range(2):
                            h = fb * 2 + hh
                            nc.tensor.matmul(
                                cts[hh][:],
                                V_t[j][:, h * 65:(h + 1) * 65],
                                pts[j][:, hh * 512:(hh + 1) * 512],
                                start=(j == 0), stop=(j == NT - 1))

                    def scores(sb, fb=fb, qw=qw):
                        st = psT.tile([128, 1024], F32, tag="st", name="st")
                        sw = slice(sb * 128, (sb + 1) * 128)
                        for hh in range(2):
                            ro = hh * 64
                            nc.tensor.matmul(
                                st[:, hh * 512:(hh + 1) * 512],
                                KT[fb][ro:ro + 64, sw],
                                QT[fb][ro:ro + 64, qw],
                                start=True, stop=True)
                        return st

                    # software pipeline: exp(sb) is emitted before
                    # scores(sb+1) so the PE never delays the ACT chain;
                    # fillers go last in each step.
                    sts = {0: scores(0)}
                    for sb in range(NT):
                        pt = ptp.tile([128, 1024], F16, tag="pt", name="pt")
                        pts[sb] = pt
                        nc.scalar.activation(pt[:], sts[sb][:], EXP,
                                             scale=0.125)
                        if sb + 1 < NT:
                            sts[sb + 1] = scores(sb + 1)
                        if sb > 0:
                            c_mms(sb - 1)
                        f = fl.get(sb)
                        if f is not None:
                            f()
                    c_mms(NT - 1)
                    prev = (cts, fb, qq)

            # ---- tail: last evacs + remaining output projection -----------
            # warm matmuls cover the evac DMA round-trip so the final out
            # MMs run at full clock
            keep_warm(16)
            pc, pf, pq = prev
            evac(pc[0], pf, 0, pq)
            evac(pc[1], pf, 1, pq)
            out_unit(12)
            out_unit(13, eng=nc.scalar)
            out_unit(14)
            out_unit(15, eng=nc.scalar)

    _split_waits(nc)
    return nc


_NC = None


def _get_nc():
    global _NC
    if _NC is None:
        _NC = _build_program()
    return _NC


def _shard_inputs(x, Wq, bq, Wk, bk, Wv, bv, Wo):
    xTs = [np.ascontiguousarray(x[b].T).astype(np.float16) for b in range(2)]
    in_maps = []
    for core in range(8):
        b, g = divmod(core, 4)
        lo = g * GF
        in_maps.append({
            "xT": xTs[b],
            "Wq": np.ascontiguousarray(Wq[:, lo:lo + GF]).astype(np.float16),
            "Wk": np.ascontiguousarray(Wk[:, lo:lo + GF]).astype(np.float16),
            "Wv": np.ascontiguousarray(Wv[:, lo:lo + GF]).astype(np.float16),
            "Wo": np.ascontiguousarray(Wo[lo:lo + GF, :]).astype(np.float16),
            "bq": np.ascontiguousarray(bq[lo:lo + GF].reshape(GF, 1)),
            "bk": np.ascontiguousarray(bk[lo:lo + GF].reshape(GF, 1)),
            "bvr": np.ascontiguousarray(bv[lo:lo + GF].reshape(1, GF)).astype(
                np.float16),
        })
    return in_maps


def run(inputs, trace=False, trace_kwargs=None):
    """Run the kernel; returns (output [2,2048,1024] f32, BassKernelResults)."""
    inputs = {k: np.asarray(v, dtype=np.float32) for k, v in inputs.items()}
    in_maps = _shard_inputs(
        inputs["x"], inputs["Wq"], inputs["bq"], inputs["Wk"], inputs["bk"],
        inputs["Wv"], inputs["bv"], inputs["Wo"])
    nc = _get_nc()
    res = run_bass_kernel_spmd(
        nc, in_maps, list(range(8)), trace=trace, **(trace_kwargs or {}))
    bo = inputs["bo"]
    out = np.empty((2, T, D), dtype=np.float32)
    for b in range(2):
        acc = None
        for g in range(4):
            part = res.results[4 * b + g]["out"].astype(np.float32)
            acc = part if acc is None else acc + part
        out[b] = acc + bo[None, :]
    return out, res


def kernel(**inputs):
    out, _ = run(inputs, trace=False)
    return out


# revision 35
# speedup vs baseline: 1.0327x; 1.0327x over previous
"""Multi-head attention (b=2, t=2048, d=1024, h=16, hd=64) on 8 trn2 NeuronCores.

Sharding: core c = 4*b + g handles batch b and head-group g (4 heads,
feature columns [g*256, (g+1)*256)). QKV weights column-sharded, Wo
row-sharded (Megatron); each core returns ONE fp16 partial [2048, 1024]
(both 128-row Wo chunks accumulated on-device in PSUM); the host sums
the 4 group partials per batch and adds bo.

Datapath: fp16 operands everywhere on the PE (x, Wq/Wk/Wv, Q^T, K^T, V,
probs, ctx, Wo) with fp32 PSUM accumulation. Softmax skips
max-subtraction: scores are q.k/8 with q,k ~ N(0,1), far inside exp's
range.

Schedule: 8 attention phases of (fb, q-quarter): the two 64-row heads of
feature-block fb are processed CONCURRENTLY via PE row-tiles (partitions
0-63 / 64-127 -> tile_position (0,0)/(64,0)) into the two 512-wide
halves of one [128,1024] PSUM tile, so each (sb, phase) step is a single
[128,1024] exp ACTIVATE. ACT(exp) is the roofline (~147us); all other PE
work (Q/K projections, token-major V projection, fused fb0+fb1 output
projection) is interleaved as fillers to keep the PE HAM clock-gate warm
and hidden under ACT. Softmax denominators are broadcast by a K=1
matmul and inverted with the fast custom-DVE reciprocal.
"""

import numpy as np

import concourse.bass as bass
import concourse.mybir as mybir
import concourse.tile as tile
from concourse.bass_utils import run_bass_kernel_spmd

F32 = mybir.dt.float32
F32R = mybir.dt.float32r
F16 = mybir.dt.float16
EXP = mybir.ActivationFunctionType.Exp

T = 2048          # tokens per batch
D = 1024          # model dim
HG = 4            # heads per core
HD = 64           # head dim
GF = HG * HD      # 256 features per head-group
VW = HG * (HD + 1)  # 260: V columns + a ones column per head
NT = T // 128     # 16 token blocks

MAX_WAITS = 1


def _split_waits(nc):
    """walrus in this container allows only one sync-wait per instruction;
    hoist extras onto same-engine NoOps immediately before the offender."""
    for f in nc.m.functions:
        for blk in f.blocks:
            insts = list(blk.instructions)
            new, changed = [], False
            for ins in insts:
                si = ins.sync_info
                waits = list(si.on_wait) if si and si.on_wait else []
                if len(waits) > MAX_WAITS:
                    changed = True
                    extra, keep = waits[:-MAX_WAITS], waits[-MAX_WAITS:]
                    for i in range(0, len(extra), MAX_WAITS):
                        new.append(mybir.InstNoOp(
                            name=f"{ins.name}-wsplit{i}",
                            engine=ins.engine,
                            sync_info=mybir.SyncInfo(
                                on_wait=extra[i:i + MAX_WAITS], on_update=[]),
                        ))
                    ins.sync_info = mybir.SyncInfo(
                        on_wait=keep,
                        on_update=list(si.on_update) if si.on_update else [])
                new.append(ins)
            if changed:
                blk.instructions = new


def _build_program():
    nc = bass.Bass("TRN2", target_bir_lowering=False, debug=False, num_devices=8)

    xT = nc.dram_tensor("xT", [D, T], F16, kind="ExternalInput")
    Wq = nc.dram_tensor("Wq", [D, GF], F16, kind="ExternalInput")
    Wk = nc.dram_tensor("Wk", [D, GF], F16, kind="ExternalInput")
    Wv = nc.dram_tensor("Wv", [D, GF], F16, kind="ExternalInput")
    Wo = nc.dram_tensor("Wo", [GF, D], F16, kind="ExternalInput")
    # biases come as rows (single-descriptor DMAs; a [128,1] column DMA
    # is 128 four-byte packets and poisons the queue) and are transposed
    # on-chip with K=1 matmuls.
    bqr = nc.dram_tensor("bqr", [1, GF], F16, kind="ExternalInput")
    bkr = nc.dram_tensor("bkr", [1, GF], F16, kind="ExternalInput")
    bvr = nc.dram_tensor("bvr", [1, GF], F16, kind="ExternalInput")
    out = nc.dram_tensor("out", [T, D], F16, kind="ExternalOutput")

    with tile.TileContext(nc) as tc:
        with (
            nc.allow_low_precision(reason="fp16 rounding is intentional"),
            tc.tile_pool(name="w", bufs=1) as wp,       # persistent tiles
            tc.tile_pool(name="xt", bufs=8) as xp,      # xT tiles
            tc.tile_pool(name="pt", bufs=4) as ptp,     # probs tiles
            tc.tile_pool(name="sg", bufs=4) as sgp,     # stg / recip tiles
            tc.tile_pool(name="ob", bufs=3) as obp,     # out staging
            tc.tile_pool(name="ps", bufs=2, space="PSUM") as psA,   # scratch
            tc.tile_pool(name="pst", bufs=2, space="PSUM") as psT,  # S tiles
            tc.tile_pool(name="psc", bufs=2, space="PSUM") as psC,  # C accum
        ):
            # ---- tiny constants (no DMA dependency) ------------------------
            ones16 = wp.tile([1, 512], F16, tag="ones16")
            nc.gpsimd.memset(ones16[:], 1.0)
            # f32r ones row at partition 0 for the recip-broadcast matmul
            ones_f = wp.tile([1, 128], F32, tag="ones_f")
            nc.gpsimd.memset(ones_f[:], 1.0)
            onesr = wp.tile([1, 128], F32R, tag="onesr")
            nc.vector.tensor_copy(onesr[:], ones_f[:])
            wscr = wp.tile([1, 512], F16, tag="wscr")

            # warm the PE HAM clock + load the exp ACT table while input
            # DMAs land. K=1 matmuls do NOT drive the HAM activity monitor
            # past its un-throttle threshold — use full K=128 matmuls on a
            # memset tile.
            wbig = wp.tile([128, 512], F16, tag="wbig")
            nc.gpsimd.memset(wbig[:], 0.01)
            wps = psA.tile([128, 512], F32, tag="sp", name="warm")
            for i in range(36):
                nc.tensor.matmul(wps[:], wbig[:, 0:128], wbig[:],
                                 start=True, stop=True)
            nc.scalar.activation(wscr[:], wps[0:1, :], EXP, scale=0.0001)

            def keep_warm(n, pool=None):
                """dependency-free K=128 matmuls that keep the HAM clock at
                8/8 through stretches where real PE work is blocked on DMAs
                or evacs."""
                if pool is None:
                    w = psT.tile([128, 1024], F32, tag="st", name="warm2")
                    w = w[:, 0:512]
                else:
                    w = pool.tile([128, 512], F32, tag="sp", name="warm2")[:]
                for i in range(n):
                    nc.tensor.matmul(w, wbig[:, 0:128], wbig[:],
                                     start=True, stop=True)

            # ---- input DMAs: few big transfers, split across the two
            # HWDGE queues (sync + scalar) and gpsimd SWDGE ----------------
            def chunked(dst, src, c, f, eng, half=None):
                """load src [(c p), f] into dst [128, c*f] chunk-major."""
                o = dst[:].rearrange("p (c f) -> p c f", c=c, f=f)
                i = src[:, :].rearrange("(c p) f -> p c f", c=c, p=128)
                if half is not None:
                    h0 = half * (f // 2)
                    o = o[:, :, h0:h0 + f // 2]
                    i = i[:, :, h0:h0 + f // 2]
                eng.dma_start(o, i)

            bqr_t = wp.tile([1, GF], F16, tag="bqr")
            nc.sync.dma_start(bqr_t[:], bqr[:, :])
            bkr_t = wp.tile([1, GF], F16, tag="bkr")
            nc.scalar.dma_start(bkr_t[:], bkr[:, :])

            Wkb = wp.tile([128, 8 * GF], F16, tag="wkb")
            chunked(Wkb, Wk, 8, GF, nc.scalar)
            # x arrives in token quarters, split across the two HWDGE
            # queues so projection units start as their quarter lands; Q/V
            # weights ride the gpsimd SWDGE queue.
            xb = wp.tile([128, 8 * T], F16, tag="xb")
            xbv = xb[:].rearrange("p (c f) -> p c f", c=8, f=T)
            xiv = xT[:, :].rearrange("(c p) f -> p c f", c=8, p=128)
            for qtr, eng in ((0, nc.sync), (1, nc.sync),
                             (2, nc.scalar), (3, nc.scalar)):
                qs = slice(qtr * 512, (qtr + 1) * 512)
                eng.dma_start(xbv[:, :, qs], xiv[:, :, qs])
            bv_t = wp.tile([1, GF], F16, tag="bvr")
            nc.gpsimd.dma_start(bv_t[:], bvr[:, :])
            Wvb = wp.tile([128, 8 * GF], F16, tag="wvb")
            chunked(Wvb, Wv, 8, GF, nc.gpsimd)
            Wqb = wp.tile([128, 8 * GF], F16, tag="wqb")
            chunked(Wqb, Wq, 8, GF, nc.gpsimd)
            Wob = wp.tile([128, 2 * D], F16, tag="wob")
            chunked(Wob, Wo, 2, D, nc.gpsimd)

            Wq_t = [Wqb[:, dc * GF:(dc + 1) * GF] for dc in range(8)]
            Wk_t = [Wkb[:, dc * GF:(dc + 1) * GF] for dc in range(8)]
            Wv_t = [Wvb[:, dc * GF:(dc + 1) * GF] for dc in range(8)]
            Wo_t = [Wob[:, fb * D:(fb + 1) * D] for fb in range(2)]
            xT_t = [xb[:, dc * T:(dc + 1) * T] for dc in range(8)]

            # ---- persistent SBUF tensors -----------------------------------
            QT = [wp.tile([128, T], F16, tag=f"qt{fb}", name=f"qt{fb}")
                  for fb in range(2)]
            KT = [wp.tile([128, T], F16, tag=f"kt{fb}", name=f"kt{fb}")
                  for fb in range(2)]
            # V in token-major blocks [128 tokens, 4*(64+1)]; col h*65+64
            # is a ones column so the ct psum row 64 is the softmax denom.
            V_t = [wp.tile([128, VW], F16, tag=f"v{tb}", name=f"v{tb}")
                   for tb in range(NT)]
            for tb in range(NT):
                vv = V_t[tb][:].rearrange("p (h w) -> p h w", h=HG, w=HD + 1)
                nc.gpsimd.memset(vv[:, :, HD:HD + 1], 1.0)
            # normalized ctx, feature-major per fb: [128 feat, 2048 q] fp16
            CTn = [wp.tile([128, T], F16, tag=f"ctn{fb}", name=f"ctn{fb}")
                   for fb in range(2)]

            # transpose the q/k bias rows into per-partition columns
            bq_t, bk_t = [], []
            btr = psA.tile([128, 512], F32, tag="sp", name="btr")
            for i, (row, lst) in enumerate(((bqr_t, bq_t), (bkr_t, bk_t))):
                for fb in range(2):
                    c = 2 * i + fb
                    nc.tensor.matmul(
                        btr[:, c:c + 1], row[0:1, fb * 128:(fb + 1) * 128],
                        ones16[0:1, 0:1], start=True, stop=True)
                    b = wp.tile([128, 1], F32, tag=f"b{i}{fb}",
                                name=f"b{i}{fb}")
                    nc.vector.tensor_copy(b[:], btr[:, c:c + 1])
                    lst.append(b)

            # ---- helpers ---------------------------------------------------
            def qk_proj(w_t, b_t, dst, fb, tck, half=None, state={}):
                """project feature block fb of Q or K for token chunk tck.
                half=0/1 runs only dc 0-3 / 4-7 so a unit can be split
                across two filler slots (keeps each slot under the ACT
                budget)."""
                if half in (None, 0):
                    state[id(dst), fb, tck] = p = psA.tile(
                        [128, 512], F32, tag="sp", name="sp")
                else:
                    p = state.pop((id(dst), fb, tck))
                dcs = {None: range(8), 0: range(4), 1: range(4, 8)}[half]
                for dc in dcs:
                    nc.tensor.matmul(
                        p[:],
                        w_t[dc][:, fb * 128:(fb + 1) * 128],
                        xT_t[dc][:, tck * 512:(tck + 1) * 512],
                        start=(dc == 0), stop=(dc == 7))
                if half in (None, 1):
                    nc.vector.tensor_scalar_add(
                        dst[fb][:, tck * 512:(tck + 1) * 512], p[:], b_t[fb])

            def v_unit(tb):
                """token-major V projection for token block tb (+bias)."""
                p = psA.tile([128, 512], F32, tag="sp", name="sp")
                nc.tensor.matmul(p[:, 0:GF], ones16[0:1, 0:128], bv_t[0:1, :],
                                 start=True, stop=False)
                for dc in range(8):
                    nc.tensor.matmul(
                        p[:, 0:GF],
                        xT_t[dc][:, tb * 128:(tb + 1) * 128],
                        Wv_t[dc],
                        start=False, stop=(dc == 7))
                vv = V_t[tb][:].rearrange("p (h w) -> p h w", h=HG, w=HD + 1)
                pv = p[:, 0:GF].rearrange("p (h w) -> p h w", h=HG, w=HD)
                nc.vector.tensor_copy(vv[:, :, 0:HD], pv[:])

            def out_unit(tb, eng=None, nck_only=None, state={}):
                """fused fb0+fb1 output projection for token block tb.
                nck_only=0/1 runs a single 512-wide half per call."""
                if nck_only in (None, 0):
                    state[tb] = o = obp.tile([128, D], F16, tag="o", name="o")
                else:
                    o = state.pop(tb)
                for nck in ((0, 1) if nck_only is None else (nck_only,)):
                    p = psA.tile([128, 512], F32, tag="sp", name="sp")
                    for fb in range(2):
                        nc.tensor.matmul(
                            p[:],
                            CTn[fb][:, tb * 128:(tb + 1) * 128],
                            Wo_t[fb][:, nck * 512:(nck + 1) * 512],
                            start=(fb == 0), stop=(fb == 1))
                    nc.vector.tensor_copy(o[:, nck * 512:(nck + 1) * 512], p[:])
                if nck_only in (None, 1):
                    (eng or nc.sync).dma_start(
                        out[tb * 128:(tb + 1) * 128, :], o[:])

            def evac_a(ct):
                """stage ct [65, 512] to SBUF and start the reciprocal of
                its denominator row: DMA-transpose to [128, 4] so the slow
                DVE reciprocal runs 4 elems/lane instead of 512, then
                transpose the reciprocals back. No PE instructions — safe
                to schedule right at a phase boundary."""
                stg = sgp.tile([65, 512], F32R, tag="stg", name="stg")
                nc.vector.tensor_copy(stg[:], ct[:])
                dT = sgp.tile([128, 4], F32, tag="dT", name="dT")
                nc.sync.dma_start(
                    dT[:],
                    stg[64:65, :].bitcast(F32).rearrange(
                        "p (a b) -> p a b", a=128, b=4))
                rT = sgp.tile([128, 4], F32, tag="rT", name="rT")
                nc.vector.reciprocal(rT[:], dT[:])
                rrow = sgp.tile([1, 512], F32R, tag="rrow", name="rrow")
                nc.sync.dma_start(
                    rrow[:].bitcast(F32).rearrange(
                        "p (a b) -> p a b", a=128, b=4),
                    rT[:])
                return stg, rrow

            def evac_b(stg, rrow, fb, hh, qq):
                """broadcast the reciprocals to 64 partitions (K=1 matmul)
                and write normalized ctx into CTn. Scheduled a few slots
                after evac_a so the DMA round-trip never stalls the PE
                FIFO."""
                rp = psA.tile([128, 512], F32, tag="sp", name="sp")
                nc.tensor.matmul(rp[:], onesr[0:1, :], rrow[0:1, :],
                                 start=True, stop=True)
                nc.vector.tensor_mul(
                    CTn[fb][hh * 64:(hh + 1) * 64,
                            qq * 512:(qq + 1) * 512],
                    stg[0:64, :], rp[0:64, :])

            # ---- pre-phase: only what attention phase (0,0) needs -- all
            # of K(fb0), Q(fb0) quarter 0, and the first V blocks -- ordered
            # to match the token-quarter DMA arrival. Dependency-free warm
            # matmuls keep the PE clock at 8/8 across the DMA-wait bubbles.
            qk_proj(Wk_t, bk_t, KT, 0, 0)
            keep_warm(3)
            v_unit(0)
            v_unit(1)
            qk_proj(Wk_t, bk_t, KT, 0, 1)
            keep_warm(3)
            v_unit(2)
            v_unit(3)
            qk_proj(Wk_t, bk_t, KT, 0, 2)
            keep_warm(3)
            v_unit(4)
            v_unit(5)
            qk_proj(Wk_t, bk_t, KT, 0, 3)
            v_unit(6)
            v_unit(7)
            qk_proj(Wq_t, bq_t, QT, 0, 0)

            # ---- attention: 2 fb x 4 q-quarters, 2 heads row-tiled --------
            # fillers keyed by (fb, qq): {slot: callable}. Remaining
            # projection work is split into half-units (4 MMs) and the
            # output projection into nck-halves (2 MMs) so no slot blows
            # the ~1.15us ACT budget (the scores lookahead is only 1 deep,
            # so one oversized filler slot puts a bubble in the exp chain).
            # Half-unit pairs share a psA tile across two slots; pairs are
            # spaced so the 2-buffer "sp" ring never holds 3 live tiles.
            # The previous phase's evacs split: evac_a (no PE work) at
            # slots 1,3; evac_b (broadcast MM) at 5,6 after the DMA
            # round-trip is done.
            def qk2(w_t, b_t, dst, fb, tck, slots):
                return {s: (lambda h=h: qk_proj(w_t, b_t, dst, fb, tck, h))
                        for h, s in enumerate(slots)}

            def out2(tb, slots):
                return {s: (lambda n=n, tb=tb: out_unit(tb, nck_only=n))
                        for n, s in enumerate(slots)}

            def merge(*ds):
                r = {}
                for dd in ds:
                    for k, v in dd.items():
                        assert k not in r, f"slot clash {k}"
                        r[k] = v
                return r

            extra = {
                (0, 0): merge({2 * j: (lambda j=j: v_unit(j + 8))
                               for j in range(8)},
                              qk2(Wq_t, bq_t, QT, 0, 1, (1, 3)),
                              qk2(Wq_t, bq_t, QT, 0, 2, (5, 7))),
                (0, 1): merge(qk2(Wk_t, bk_t, KT, 1, 0, (0, 2)),
                              qk2(Wq_t, bq_t, QT, 0, 3, (7, 9)),
                              qk2(Wq_t, bq_t, QT, 1, 0, (11, 13))),
                (0, 2): merge(qk2(Wk_t, bk_t, KT, 1, 1, (0, 2)),
                              qk2(Wk_t, bk_t, KT, 1, 2, (7, 9)),
                              qk2(Wk_t, bk_t, KT, 1, 3, (11, 13))),
                (0, 3): merge(qk2(Wq_t, bq_t, QT, 1, 1, (0, 2)),
                              qk2(Wq_t, bq_t, QT, 1, 2, (7, 9)),
                              qk2(Wq_t, bq_t, QT, 1, 3, (11, 13))),
                (1, 1): merge(out2(0, (7, 8)), out2(1, (9, 10)),
                              out2(2, (11, 12)), out2(3, (13, 14))),
                (1, 2): merge(out2(4, (7, 8)), out2(5, (9, 10)),
                              out2(6, (11, 12)), out2(7, (13, 14))),
                (1, 3): merge(out2(8, (7, 8)), out2(9, (9, 10)),
                              out2(10, (11, 12)), out2(11, (13, 14))),
            }

            phases = [(fb, qq) for fb in range(2) for qq in range(4)]

            def scores(fb, qq, sb):
                st = psT.tile([128, 1024], F32, tag="st", name="st")
                sw = slice(sb * 128, (sb + 1) * 128)
                qw = slice(qq * 512, (qq + 1) * 512)
                for hh in range(2):
                    ro = hh * 64
                    nc.tensor.matmul(
                        st[:, hh * 512:(hh + 1) * 512],
                        KT[fb][ro:ro + 64, sw],
                        QT[fb][ro:ro + 64, qw],
                        start=True, stop=True)
                return st

            prev = None
            next_st = None
            for pi, (fb, qq) in enumerate(phases):
                cts = [psC.tile([65, 512], F32, tag="ct", name=f"ct{hh}")
                       for hh in range(2)]
                pts = {}
                fl = dict(extra.get((fb, qq), {}))
                if prev is not None:
                    pc, pf, pq = prev
                    ev = {}
                    fl[1] = (lambda a=fl.get(1), pc=pc, ev=ev:
                             (ev.update(h0=evac_a(pc[0])), a and a()))
                    fl[3] = (lambda a=fl.get(3), pc=pc, ev=ev:
                             (ev.update(h1=evac_a(pc[1])), a and a()))
                    fl[5] = (lambda a=fl.get(5), ev=ev, pf=pf, pq=pq:
                             (evac_b(*ev["h0"], pf, 0, pq), a and a()))
                    fl[6] = (lambda a=fl.get(6), ev=ev, pf=pf, pq=pq:
                             (evac_b(*ev["h1"], pf, 1, pq), a and a()))

                def c_mms(j, cts=cts, fb=fb, pts=pts):
                    for hh in range(2):
                        h = fb * 2 + hh
                        nc.tensor.matmul(
                            cts[hh][:],
                            V_t[j][:, h * 65:(h + 1) * 65],
                            pts[j][:, hh * 512:(hh + 1) * 512],
                            start=(j == 0), stop=(j == NT - 1))

                # software pipeline: exp(sb) is emitted before scores(sb+1)
                # so the PE never delays the ACT chain; the NEXT phase's
                # scores(0) is emitted at sb=15 so phase boundaries don't
                # bubble either; fillers go last in each step.
                sts = {0: next_st if next_st is not None
                       else scores(fb, qq, 0)}
                for sb in range(NT):
                    pt = ptp.tile([128, 1024], F16, tag="pt", name="pt")
                    pts[sb] = pt
                    nc.scalar.activation(pt[:], sts[sb][:], EXP, scale=0.125)
                    if sb + 1 < NT:
                        sts[sb + 1] = scores(fb, qq, sb + 1)
                    elif pi + 1 < len(phases):
                        next_st = scores(*phases[pi + 1], 0)
                    else:
                        next_st = None
                    if sb > 0:
                        c_mms(sb - 1)
                    f = fl.get(sb)
                    if f is not None:
                        f()
                c_mms(NT - 1)
                prev = (cts, fb, qq)

            # ---- tail: last evacs + remaining output projection -----------
            # warm matmuls cover the evac DMA round-trip so the final out
            # MMs run at full clock
            keep_warm(14)
            pc, pf, pq = prev
            e0 = evac_a(pc[0])
            e1 = evac_a(pc[1])
            keep_warm(4)
            evac_b(*e0, pf, 0, pq)
            evac_b(*e1, pf, 1, pq)
            out_unit(12)
            out_unit(13, eng=nc.scalar)
            out_unit(14)
            out_unit(15, eng=nc.scalar)

    _split_waits(nc)
    return nc


_NC = None


def _get_nc():
    global _NC
    if _NC is None:
        _NC = _build_program()
    return _NC


def _shard_inputs(x, Wq, bq, Wk, bk, Wv, bv, Wo):
    xTs = [np.ascontiguousarray(x[b].T).astype(np.float16) for b in range(2)]
    in_maps = []
    for core in range(8):
        b, g = divmod(core, 4)
        lo = g * GF
        in_maps.append({
            "xT": xTs[b],
            "Wq": np.ascontiguousarray(Wq[:, lo:lo + GF]).astype(np.float16),
            "Wk": np.ascontiguousarray(Wk[:, lo:lo + GF]).astype(np.float16),
            "Wv": np.ascontiguousarray(Wv[:, lo:lo + GF]).astype(np.float16),
            "Wo": np.ascontiguousarray(Wo[lo:lo + GF, :]).astype(np.float16),
            "bqr": np.ascontiguousarray(bq[lo:lo + GF].reshape(1, GF)).astype(
                np.float16),
            "bkr": np.ascontiguousarray(bk[lo:lo + GF].reshape(1, GF)).astype(
                np.float16),
            "bvr": np.ascontiguousarray(bv[lo:lo + GF].reshape(1, GF)).astype(
                np.float16),
        })
    return in_maps


def run(inputs, trace=False, trace_kwargs=None):
    """Run the kernel; returns (output [2,2048,1024] f32, BassKernelResults)."""
    inputs = {k: np.asarray(v, dtype=np.float32) for k, v in inputs.items()}
    in_maps = _shard_inputs(
        inputs["x"], inputs["Wq"], inputs["bq"], inputs["Wk"], inputs["bk"],
        inputs["Wv"], inputs["bv"], inputs["Wo"])
    nc = _get_nc()
    res = run_bass_kernel_spmd(
        nc, in_maps, list(range(8)), trace=trace, **(trace_kwargs or {}))
    bo = inputs["bo"]
    out = np.empty((2, T, D), dtype=np.float32)
    for b in range(2):
        acc = None
        for g in range(4):
            part = res.results[4 * b + g]["out"].astype(np.float32)
            acc = part if acc is None else acc + part
        out[b] = acc + bo[None, :]
    return out, res


def kernel(**inputs):
    out, _ = run(inputs, trace=False)
    return out
